# revision 42
# baseline (speedup 1.0000x reference)
"""MLA (multi-latent attention) Trainium2 kernel, 8-core SPMD.

Sharding: tensor-parallel over heads (4 groups of 4 heads) x data-parallel
over batch (2), = 8 cores. Low-rank a-projections are replicated; q_b/kv_b
output dims and out_proj input dim are sharded by head. Each core returns a
token-major partial out-projection [n, 2048]; the host sums the 4 head-group
partials per batch element (the standard row-parallel TP unshard).

On-chip layout is feature-major ("T" = [features on partitions, tokens on
free]) so every matmul contracts over the partition dim with natural layouts.

Attention keeps keys on partitions / queries on free. Per 512-query block
all 4 heads are processed together per 128-key chunk: scoresT for head
pairs (0,1) and (2,3) land in [128,1024] PSUM tiles (head-halves in
adjacent banks), one wide exp per pair on the scalar engine, causal mask
applied post-exp as a 0/1 DVE multiply, softmax denominators accumulated in
bf16 on DVE, and the attn@V matmuls are delayed one key-chunk so the PE
never waits on the exp. The two 64-wide rope matmuls of a head pair occupy
disjoint PE row groups and run concurrently. The out-projection of query
block qb-1 is interleaved into block qb's key loop and DMA'd to DRAM
straight out of PSUM. No max subtraction (logits for this distribution are
O(10), far from fp32 exp overflow).
"""

from contextlib import ExitStack

import numpy as np
import ml_dtypes

import concourse.bacc as bacc
import concourse.mybir as mybir
from concourse.tile import TileContext
from concourse import bass_utils

BF16 = mybir.dt.bfloat16
F32 = mybir.dt.float32
NPBF16 = ml_dtypes.bfloat16

EMBED = 2048
HEADS = 16
NOPE = 128
VDIM = 128
ROPE = 64
Q_HEAD = NOPE + ROPE  # 192
KV_RANK = 512
BASE = 10000.0
SCALE = 1.0 / float(np.sqrt(Q_HEAD))

NH = 4          # heads per core
KC = EMBED // 128   # 16 k-chunks of the embedding dim
RC = KV_RANK // 128  # 4 k-chunks of the kv rank


def _emit(nc, n):
    """Trace the per-core kernel (same program on all 8 cores)."""
    TC = n // 512   # token chunks of 512
    NT = n // 128   # token chunks of 128
    AF = mybir.ActivationFunctionType
    OP = mybir.AluOpType

    # ---- DRAM I/O ----
    d_x = nc.dram_tensor("xT", [128, TC, KC, 512], BF16, kind="ExternalInput")
    d_wqa = nc.dram_tensor("wqa", [128, KC, 512], BF16, kind="ExternalInput")
    d_wkva = nc.dram_tensor("wkva", [128, KC, 576], BF16, kind="ExternalInput")
    d_wqb = nc.dram_tensor("wqb", [128, RC, NH * Q_HEAD], BF16, kind="ExternalInput")
    d_wk = nc.dram_tensor("wk", [128, RC, NH * NOPE], BF16, kind="ExternalInput")
    d_wv = nc.dram_tensor("wv", [128, RC, NH * VDIM], BF16, kind="ExternalInput")
    d_wout = nc.dram_tensor("wout", [128, NH, EMBED], BF16, kind="ExternalInput")
    d_cos = nc.dram_tensor("cosd", [128, n], F32, kind="ExternalInput")
    d_sin = nc.dram_tensor("sind", [128, n], F32, kind="ExternalInput")
    d_mask = nc.dram_tensor("maskd", [128, 4, 1024], BF16, kind="ExternalInput")
    d_rotp = nc.dram_tensor("rotp", [128, 128], BF16, kind="ExternalInput")
    d_rotk = nc.dram_tensor("rotk", [64, 128], BF16, kind="ExternalInput")
    d_eyek = nc.dram_tensor("eyek", [64, 128], BF16, kind="ExternalInput")
    d_ones = nc.dram_tensor("onesd", [128, 128], BF16, kind="ExternalInput")
    d_out = nc.dram_tensor("out", [n, EMBED], F32, kind="ExternalOutput")

    with TileContext(nc) as tc, ExitStack() as st:
        # ---- attention-phase persistent tiles (allocated for whole kernel,
        # prefetched during phase 1) ----
        attn_p = st.enter_context(tc.tile_pool(name="attn_p", bufs=1))
        t_qn = attn_p.tile([128, NH, n], BF16)
        t_qpe = attn_p.tile([128, 2, n], BF16)
        t_kn = attn_p.tile([128, NH, n], BF16)
        t_kpe = attn_p.tile([128, n], BF16)
        t_v = attn_p.tile([128, NT, NH * VDIM], BF16)

        # ---- mid pool: phase-1/2 intermediates, freed before attention ----
        mid = st.enter_context(tc.tile_pool(name="mid", bufs=1))
        t_qa = mid.tile([128, RC, n], BF16)
        t_ckv = mid.tile([128, RC, n], BF16)
        t_kpr = mid.tile([64, n], BF16)  # raw k_pe (pre-rope)
        t_cos = mid.tile([128, n], F32)
        t_sin = mid.tile([128, n], F32)
        t_rotp = mid.tile([128, 128], BF16)
        t_rotk = mid.tile([64, 128], BF16)
        t_eyek = mid.tile([64, 128], BF16)
        t_wqb = mid.tile([128, RC, NH * Q_HEAD], BF16)
        t_wk = mid.tile([128, RC, NH * NOPE], BF16)
        t_wv = mid.tile([128, RC, NH * VDIM], BF16)

        # ================= phase 1: qa = x@Wqa, ckv = x@Wkva =============
        ps12_cm = tc.tile_pool(name="ps12", bufs=1, space="PSUM")
        ps12 = ps12_cm.__enter__()
        with tc.tile_pool(name="ph1", bufs=1) as ph1:
            rot_i = [0]

            def ps_any(name):
                tag = "pa" if rot_i[0] % 8 < 4 else "pb"
                rot_i[0] += 1
                return ps12.tile([128, 512], F32, tag=tag, bufs=4, name=name)

            t_wqa = ph1.tile([128, KC, 512], BF16)
            t_wkva = ph1.tile([128, KC, 576], BF16)
            # critical-path loads first, chunked so the first matmuls can
            # start before the full tensors land; spread over engine queues
            # PE pre-warm: dummy matmuls on a zeroed tile keep the HAM
            # clock-gate at full speed until the first input DMAs land
            # (~14us); their accumulation groups are reset by the real
            # phase-1 matmuls (start=True).
            t_warm = ph1.tile([128, 512], BF16)
            nc.vector.memset(t_warm, 0)
            ps_w = ps_any("ps_w")
            for _ in range(50):
                nc.tensor.matmul(ps_w, t_warm[:, 0:128], t_warm,
                                 start=True, stop=True)
            t_xc = [ph1.tile([128, KC, 512], BF16, tag="tx", bufs=2, name="tx")]
            for kk in range(4):
                kks = slice(4 * kk, 4 * kk + 4)
                nc.sync.dma_start(out=t_xc[0][:, kks], in_=d_x.ap()[:, 0, kks])
                nc.scalar.dma_start(out=t_wqa[:, kks], in_=d_wqa.ap()[:, kks])
            for kk in range(2):
                kks = slice(8 * kk, 8 * kk + 8)
                nc.gpsimd.dma_start(out=t_wkva[:, kks], in_=d_wkva.ap()[:, kks])
            nc.gpsimd.dma_start(out=t_wqb, in_=d_wqb.ap())
            nc.gpsimd.dma_start(out=t_wk, in_=d_wk.ap())
            nc.gpsimd.dma_start(out=t_wv, in_=d_wv.ap())
            nc.gpsimd.dma_start(out=t_cos, in_=d_cos.ap())
            nc.gpsimd.dma_start(out=t_sin, in_=d_sin.ap())
            nc.gpsimd.dma_start(out=t_rotp, in_=d_rotp.ap())
            nc.gpsimd.dma_start(out=t_rotk, in_=d_rotk.ap())
            nc.gpsimd.dma_start(out=t_eyek, in_=d_eyek.ap())

            for t in range(TC):
                ts = slice(t * 512, (t + 1) * 512)
                if t + 1 < TC:
                    t_xc.append(
                        ph1.tile([128, KC, 512], BF16, tag="tx", bufs=2, name="tx")
                    )
                    nc.sync.dma_start(out=t_xc[t + 1], in_=d_x.ap()[:, t + 1])
                t_x = t_xc[t]
                # 8 concurrent accumulation groups (one psum bank each) so an
                # arriving x k-piece immediately feeds 8 matmuls
                pss = [ps_any(f"pg{g}") for g in range(8)]
                for k in range(KC):
                    for g in range(8):
                        w = (
                            t_wqa[:, k, g * 128 : (g + 1) * 128]
                            if g < 4
                            else t_wkva[:, k, (g - 4) * 128 : (g - 3) * 128]
                        )
                        nc.tensor.matmul(
                            pss[g],
                            w,
                            t_x[:, k, :],
                            start=(k == 0),
                            stop=(k == KC - 1),
                        )
                for g in range(8):
                    dst = t_qa[:, g, ts] if g < 4 else t_ckv[:, g - 4, ts]
                    if g % 2 == 0:
                        nc.scalar.copy(dst, pss[g])
                    else:
                        nc.vector.tensor_copy(dst, pss[g])
                # k_pe chunk (64 wide)
                ps = ps_any("ps3")
                for k in range(KC):
                    nc.tensor.matmul(
                        ps[:64],
                        t_wkva[:, k, 512:576],
                        t_x[:, k, :],
                        start=(k == 0),
                        stop=(k == KC - 1),
                    )
                nc.scalar.copy(t_kpr[:, ts], ps[:64])

        wop = st.enter_context(tc.tile_pool(name="wop", bufs=1))
        t_wout = wop.tile([128, NH, EMBED], BF16)
        t_mask = wop.tile([128, 4, 1024], BF16)
        t_ones = wop.tile([128, 128], BF16)
        nc.scalar.dma_start(out=t_wout, in_=d_wout.ap())
        nc.scalar.dma_start(out=t_mask, in_=d_mask.ap())
        nc.scalar.dma_start(out=t_ones, in_=d_ones.ap())

        # ==== phase 2: q/k/v projections + rope, interleaved per chunk ====
        with tc.tile_pool(name="ropep", bufs=4) as rp:
            rot_i = [0]

            def ps_any2(name):
                tag = "pa" if rot_i[0] % 8 < 4 else "pb"
                rot_i[0] += 1
                return ps12.tile([128, 512], F32, tag=tag, bufs=4, name=name)

            def rope_q(g, t):
                ts = slice(t * 512, (t + 1) * 512)
                pr = ps_any2("prq")
                nc.tensor.matmul(pr, t_rotp, t_qpe[:, g, ts])
                tt1 = rp.tile([128, 512], F32, tag="tt1")
                tt2 = rp.tile([128, 512], F32, tag="tt2")
                nc.vector.tensor_tensor(tt1, pr, t_sin[:, ts], op=OP.mult)
                nc.vector.tensor_tensor(
                    tt2, t_qpe[:, g, ts], t_cos[:, ts], op=OP.mult
                )
                nc.vector.tensor_tensor(t_qpe[:, g, ts], tt1, tt2, op=OP.add)

            def rope_k(t):
                ts = slice(t * 512, (t + 1) * 512)
                pr = ps_any2("prk")
                pd = ps_any2("pdk")
                nc.tensor.matmul(pr, t_rotk, t_kpr[:, ts])
                nc.tensor.matmul(pd, t_eyek, t_kpr[:, ts])
                tt1 = rp.tile([128, 512], F32, tag="tt1")
                tt2 = rp.tile([128, 512], F32, tag="tt2")
                nc.vector.tensor_tensor(tt1, pr, t_sin[:, ts], op=OP.mult)
                nc.vector.tensor_tensor(tt2, pd, t_cos[:, ts], op=OP.mult)
                nc.vector.tensor_tensor(t_kpe[:, ts], tt1, tt2, op=OP.add)

            for t in range(TC):
                ts = slice(t * 512, (t + 1) * 512)
                for m in range(6):  # q: 4 nope chunks + 2 pe chunks
                    ps = ps_any2("psq")
                    for k in range(RC):
                        nc.tensor.matmul(
                            ps,
                            t_wqb[:, k, m * 128 : (m + 1) * 128],
                            t_qa[:, k, ts],
                            start=(k == 0),
                            stop=(k == RC - 1),
                        )
                    if m < 4:
                        nc.scalar.copy(t_qn[:, m, ts], ps)
                    else:
                        nc.scalar.copy(t_qpe[:, m - 4, ts], ps)
                rope_q(0, t)
                rope_q(1, t)
                for m in range(4):  # k_nope
                    ps = ps_any2("psk")
                    for k in range(RC):
                        nc.tensor.matmul(
                            ps,
                            t_wk[:, k, m * 128 : (m + 1) * 128],
                            t_ckv[:, k, ts],
                            start=(k == 0),
                            stop=(k == RC - 1),
                        )
                    nc.scalar.copy(t_kn[:, m, ts], ps)
                rope_k(t)
                for mt in range(4 * t, 4 * t + 4):  # v, token-major
                    ps = ps_any2("psv")
                    for k in range(RC):
                        nc.tensor.matmul(
                            ps,
                            t_ckv[:, k, mt * 128 : (mt + 1) * 128],
                            t_wv[:, k, :],
                            start=(k == 0),
                            stop=(k == RC - 1),
                        )
                    nc.scalar.copy(t_v[:, mt, :], ps)

        ps12_cm.__exit__(None, None, None)

        # =================== attention + out-proj ========================
        with (
            tc.tile_pool(name="ptp", bufs=1) as ptp,
            tc.tile_pool(name="accp", bufs=1) as accp,
            tc.tile_pool(name="aop", bufs=1) as aop,
            tc.tile_pool(name="smallp", bufs=1) as smallp,
            tc.tile_pool(name="otp", bufs=4) as otp,
            tc.tile_pool(name="psA", bufs=1, space="PSUM") as psA,
        ):
            def sc_tile(name):
                return psA.tile([128, 1024], F32, tag="sc", bufs=2, name=name)

            def av_tile(name):
                return psA.tile([128, 512], F32, tag="av", bufs=4, name=name)

            op_i = [0]

            def op_store(po, tok, fp):
                # copy PSUM -> SBUF (alternate DVE / ACT) then DMA out
                ot = otp.tile([128, 1024], F32, tag="ot", name="ot")
                if op_i[0] % 2 == 0:
                    nc.vector.tensor_copy(ot, po)
                else:
                    nc.scalar.copy(ot, po)
                op_i[0] += 1
                nc.sync.dma_start(
                    out=d_out.ap()[tok : tok + 128, fp * 1024 : (fp + 1) * 1024],
                    in_=ot,
                )

            ao_prev = None  # t_ao tile of the previous query block

            def emit_op_group(mt, fp, qb_tok):
                tok = qb_tok * 512 + mt * 128
                po = sc_tile("po")
                for fh in range(2):
                    f = 2 * fp + fh
                    half = slice(fh * 512, (fh + 1) * 512)
                    for h4 in range(NH):
                        nc.tensor.matmul(
                            po[:, half],
                            ao_prev[:, h4, mt * 128 : (mt + 1) * 128],
                            t_wout[:, h4, f * 512 : (f + 1) * 512],
                            start=(h4 == 0),
                            stop=(h4 == NH - 1),
                        )
                op_store(po, tok, fp)

            for qb in range(TC):
                qs = slice(qb * 512, (qb + 1) * 512)
                nkb = 4 * qb + 4
                ps_av = [av_tile(f"av{h}") for h in range(NH)]
                acc = [accp.tile([128, 1024], BF16, tag=f"acc{p}", bufs=2,
                                 name=f"acc{p}")
                       for p in range(2)]
                t_ao = aop.tile([128, NH, 512], BF16, tag="ao", bufs=3)
                op_work = (
                    [(mt, fp) for mt in range(4) for fp in range(2)]
                    if ao_prev is not None
                    else []
                )
                pending = None
                for kb in range(nkb):
                    ks = slice(kb * 128, (kb + 1) * 128)
                    diag = kb >= 4 * qb
                    r = kb - 4 * qb
                    # --- scores for 4 heads: 2 head-pair tiles ---
                    sc = [sc_tile(f"sc{p}") for p in range(2)]
                    pt = [
                        ptp.tile([128, 1024], BF16, tag=f"pt{p}", bufs=3,
                                 name=f"pt{p}")
                        for p in range(2)
                    ]
                    # for diag chunk r, queries < 128*r are fully masked:
                    # stream only columns [128r, 512). The skipped region of
                    # the psum tile holds stale data; exp reads it and the
                    # mask multiply zeroes it, so downstream stays correct.
                    qoff = 128 * r if diag else 0
                    qsub = slice(qb * 512 + qoff, (qb + 1) * 512)
                    for p in range(2):
                        for hh in range(2):
                            h = 2 * p + hh
                            half = slice(hh * 1024 // 2 + qoff, (hh + 1) * 512)
                            nc.tensor.matmul(
                                sc[p][:, half], t_kn[:, h, ks],
                                t_qn[:, h, qsub],
                                start=True, stop=False,
                                skip_group_check=True,
                            )
                        for hh in range(2):
                            ho = hh * 64
                            half = slice(hh * 512 + qoff, (hh + 1) * 512)
                            nc.tensor.matmul(
                                sc[p][:, half],
                                t_kpe[ho : ho + 64, ks],
                                t_qpe[ho : ho + 64, p, qsub],
                                start=False, stop=True,
                                skip_group_check=True,
                            )
                        nc.scalar.activation(pt[p], sc[p], AF.Exp, scale=SCALE)
                        if diag:
                            nc.vector.tensor_tensor(
                                pt[p], pt[p], t_mask[:, r, :], op=OP.mult
                            )
                        if kb == 0:
                            nc.vector.tensor_copy(acc[p], pt[p])
                        else:
                            nc.vector.tensor_tensor(
                                acc[p], acc[p], pt[p], op=OP.add
                            )
                    # --- interleaved out-proj of previous block ---
                    # two groups on alternate steps: the pair of PSUM tiles
                    # then has a two-step window before the score rotation
                    # reclaims them. Keep two in reserve to cover the den wait
                    # at the block boundary.
                    reserve = 4 if qb == TC - 1 else 2
                    if kb % 3 == 0:
                        for _ in range(2):
                            if len(op_work) > reserve:
                                mt, fp = op_work.pop(0)
                                emit_op_group(mt, fp, qb - 1)
                    # --- attn@V for the previous key chunk (1-step delay) ---
                    if pending is not None:
                        pkb, ppt = pending
                        poff = 128 * (pkb - 4 * qb) if pkb >= 4 * qb else 0
                        for h in range(NH):
                            nc.tensor.matmul(
                                ps_av[h][:, poff:512],
                                t_v[:, pkb, h * VDIM : (h + 1) * VDIM],
                                ppt[h // 2][
                                    :, (h % 2) * 512 + poff : (h % 2 + 1) * 512
                                ],
                                start=(pkb == 0),
                                stop=False,
                                skip_group_check=True,
                            )
                    pending = (kb, pt)
                # flush the last attn@V chunk
                pkb, ppt = pending
                poff = 128 * (pkb - 4 * qb) if pkb >= 4 * qb else 0
                for h in range(NH):
                    nc.tensor.matmul(
                        ps_av[h][:, poff:512],
                        t_v[:, pkb, h * VDIM : (h + 1) * VDIM],
                        ppt[h // 2][:, (h % 2) * 512 + poff : (h % 2 + 1) * 512],
                        start=(pkb == 0),
                        stop=True,
                        skip_group_check=True,
                    )
                # two leftover groups fill the den wait on the acc chain;
                # on the final block two more cover the recip+normalize wait
                for mt, fp in op_work[:2]:
                    emit_op_group(mt, fp, qb - 1)
                op_work = op_work[2:]
                # --- denominators + normalize ---
                # all-ones [128,128] stationary: every PSUM row of the den
                # matmul gets the per-query denominator (free broadcast), then
                # one reciprocal per pair lands 1/den in SBUF.
                ps_den = [sc_tile(f"psden{p}") for p in range(2)]
                if qb == 0:
                    # qb0 has no out-proj work to fill the wait on the final
                    # exp->mask->acc chain; idle here trips the HAM throttle
                    # into half-clock for the next ~7us. Keep the PE busy with
                    # throwaway matmuls whose groups the real den matmuls
                    # reset (start=True) below.
                    for p in range(2):
                        for half in (slice(0, 512), slice(512, 1024)):
                            for _ in range(2):
                                nc.tensor.matmul(
                                    ps_den[p][:, half],
                                    t_ones,
                                    t_qn[:, 0, 0:512],
                                )
                for p in range(2):
                    for half in (slice(0, 512), slice(512, 1024)):
                        nc.tensor.matmul(
                            ps_den[p][:, half],
                            t_ones,
                            acc[p][:, half],
                        )
                bc = [smallp.tile([128, 1024], F32, tag=f"bc{p}", name=f"bc{p}")
                      for p in range(2)]
                nc.vector.reciprocal_approx_fast(bc[0], ps_den[0])
                nc.vector.reciprocal_approx_fast(bc[1], ps_den[1])
                for mt, fp in op_work:  # final-block extra reserve
                    emit_op_group(mt, fp, qb - 1)
                for h in range(NH):
                    nc.vector.tensor_tensor(
                        t_ao[:, h, :],
                        ps_av[h],
                        bc[h // 2][:, (h % 2) * 512 : (h % 2 + 1) * 512],
                        op=OP.mult,
                    )
                ao_prev = t_ao
            # final block's out-projection (tail)
            for mt in range(4):
                for fp in range(2):
                    emit_op_group(mt, fp, TC - 1)
    return nc


_NC_CACHE = {}


def build_mla(n=2048):
    if n not in _NC_CACHE:
        nc = bacc.Bacc(
            "TRN2",
            target_bir_lowering=False,
            debug=False,
            enable_asserts=False,
        )
        _emit(nc, n)
        nc.compile()
        _NC_CACHE[n] = nc
    return _NC_CACHE[n]


def make_host_inputs(x, Wqa, Wqb, Wkva, Wkvb, Wout, n):
    """Build the 8 per-core input maps (host-side sharding)."""
    # rope tables
    theta = BASE ** (-2.0 * np.arange(ROPE // 2, dtype=np.float32) / ROPE)
    pos = np.arange(n, dtype=np.float32)
    ang = pos[:, None] * theta[None, :]  # [n, 32]
    cos64 = np.repeat(np.cos(ang).T, 2, axis=0).astype(np.float32)  # [64, n]
    sin64 = np.repeat(np.sin(ang).T, 2, axis=0).astype(np.float32)
    cosd = np.tile(cos64, (2, 1))  # [128, n]
    sind = np.tile(sin64, (2, 1))

    kp = np.arange(128)[:, None, None]
    r = np.arange(4)[None, :, None]
    qf = np.arange(512)[None, None, :]
    # keep-mask: 1.0 where key <= query (kept), 0.0 where future (excluded)
    keep = (qf >= r * 128 + kp).astype(NPBF16)  # [128, 4, 512]
    maskd = np.concatenate([keep, keep], axis=2)  # [128, 4, 1024] per head-pair

    rot64 = np.zeros((64, 64), np.float32)
    for i in range(32):
        rot64[2 * i + 1, 2 * i] = -1.0
        rot64[2 * i, 2 * i + 1] = 1.0
    rotp = np.zeros((128, 128), np.float32)
    rotp[:64, :64] = rot64
    rotp[64:, 64:] = rot64
    rotk = np.hstack([rot64, rot64])
    eyek = np.hstack([np.eye(64, dtype=np.float32), np.eye(64, dtype=np.float32)])

    def prelay(w, kc):
        # [kc*128, m] -> [128, kc, m] partition-major, contiguous
        return np.ascontiguousarray(
            w.reshape(kc, 128, w.shape[1]).transpose(1, 0, 2)
        ).astype(NPBF16)

    shared = {
        "wqa": prelay(Wqa, KC),
        "wkva": prelay(Wkva, KC),
        "cosd": cosd,
        "sind": sind,
        "maskd": maskd,
        "onesd": np.ones((128, 128), np.float32).astype(NPBF16),
        "rotp": rotp.astype(NPBF16),
        "rotk": rotk.astype(NPBF16),
        "eyek": eyek.astype(NPBF16),
    }
    Wqb_r = Wqb.reshape(512, HEADS, Q_HEAD)
    Wkvb_r = Wkvb.reshape(KV_RANK, HEADS, NOPE + VDIM)
    Wout_r = Wout.reshape(HEADS, VDIM, EMBED)

    in_maps = []
    TC = n // 512
    # x[be].T -> [128, TC, KC, 512]: f=(c,p), t=(tb,tt)
    xT = [
        np.ascontiguousarray(
            x[be].T.reshape(KC, 128, TC, 512).transpose(1, 2, 0, 3)
        ).astype(NPBF16)
        for be in range(x.shape[0])
    ]
    for c in range(8):
        be, hg = c // 4, c % 4
        hsel = slice(4 * hg, 4 * hg + NH)
        wqb = prelay(
            np.concatenate(
                [
                    Wqb_r[:, hsel, :NOPE].reshape(512, NH * NOPE),
                    Wqb_r[:, hsel, NOPE:].reshape(512, NH * ROPE),
                ],
                axis=1,
            ),
            RC,
        )
        in_maps.append(
            {
                **shared,
                "xT": xT[be],
                "wqb": wqb,
                "wk": prelay(Wkvb_r[:, hsel, :NOPE].reshape(512, NH * NOPE), RC),
                "wv": prelay(Wkvb_r[:, hsel, NOPE:].reshape(512, NH * VDIM), RC),
                "wout": prelay(Wout_r[hsel].reshape(NH * VDIM, EMBED), NH),
            }
        )
    return in_maps


def kernel(x, Wqa, Wqb, Wkva, Wkvb, Wout, _trace=False):
    x = np.asarray(x)
    b, n, _ = x.shape
    nc = build_mla(n)
    in_maps = make_host_inputs(
        np.asarray(x),
        np.asarray(Wqa),
        np.asarray(Wqb),
        np.asarray(Wkva),
        np.asarray(Wkvb),
        np.asarray(Wout),
        n,
    )
    res = bass_utils.run_bass_kernel_spmd(
        nc, in_maps, core_ids=list(range(8)), trace=_trace
    )
    out = np.zeros((b, n, EMBED), np.float32)
    for c in range(8):
        out[c // 4] += res.results[c]["out"]
    if _trace:
        kernel.last_results = res
    return out


# revision 43
# speedup vs baseline: 1.0065x; 1.0065x over previous
"""MLA (multi-latent attention) Trainium2 kernel, 8-core SPMD.

Sharding: tensor-parallel over heads (4 groups of 4 heads) x data-parallel
over batch (2), = 8 cores. Low-rank a-projections are replicated; q_b/kv_b
output dims and out_proj input dim are sharded by head. Each core returns a
token-major partial out-projection [n, 2048]; the host sums the 4 head-group
partials per batch element (the standard row-parallel TP unshard).

On-chip layout is feature-major ("T" = [features on partitions, tokens on
free]) so every matmul contracts over the partition dim with natural layouts.

Attention keeps keys on partitions / queries on free. Per 512-query block
all 4 heads are processed together per 128-key chunk: scoresT for head
pairs (0,1) and (2,3) land in [128,1024] PSUM tiles (head-halves in
adjacent banks), one wide exp per pair on the scalar engine, causal mask
applied post-exp as a 0/1 DVE multiply, softmax denominators accumulated in
bf16 on DVE, and the attn@V matmuls are delayed one key-chunk so the PE
never waits on the exp. The two 64-wide rope matmuls of a head pair occupy
disjoint PE row groups and run concurrently. The out-projection of query
block qb-1 is interleaved into block qb's key loop and DMA'd to DRAM
straight out of PSUM. No max subtraction (logits for this distribution are
O(10), far from fp32 exp overflow).
"""

from contextlib import ExitStack

import numpy as np
import ml_dtypes

import concourse.bacc as bacc
import concourse.mybir as mybir
from concourse.tile import TileContext
from concourse import bass_utils

BF16 = mybir.dt.bfloat16
F32 = mybir.dt.float32
NPBF16 = ml_dtypes.bfloat16

EMBED = 2048
HEADS = 16
NOPE = 128
VDIM = 128
ROPE = 64
Q_HEAD = NOPE + ROPE  # 192
KV_RANK = 512
BASE = 10000.0
SCALE = 1.0 / float(np.sqrt(Q_HEAD))

NH = 4          # heads per core
KC = EMBED // 128   # 16 k-chunks of the embedding dim
RC = KV_RANK // 128  # 4 k-chunks of the kv rank


def _emit(nc, n):
    """Trace the per-core kernel (same program on all 8 cores)."""
    TC = n // 512   # token chunks of 512
    NT = n // 128   # token chunks of 128
    AF = mybir.ActivationFunctionType
    OP = mybir.AluOpType

    # ---- DRAM I/O ----
    d_x = nc.dram_tensor("xT", [128, TC, KC, 512], BF16, kind="ExternalInput")
    d_wqa = nc.dram_tensor("wqa", [128, KC, 512], BF16, kind="ExternalInput")
    d_wkva = nc.dram_tensor("wkva", [128, KC, 576], BF16, kind="ExternalInput")
    d_wqb = nc.dram_tensor("wqb", [128, RC, NH * Q_HEAD], BF16, kind="ExternalInput")
    d_wk = nc.dram_tensor("wk", [128, RC, NH * NOPE], BF16, kind="ExternalInput")
    d_wv = nc.dram_tensor("wv", [128, RC, NH * VDIM], BF16, kind="ExternalInput")
    d_wout = nc.dram_tensor("wout", [128, NH, EMBED], BF16, kind="ExternalInput")
    d_cos = nc.dram_tensor("cosd", [128, n], F32, kind="ExternalInput")
    d_sin = nc.dram_tensor("sind", [128, n], F32, kind="ExternalInput")
    d_mask = nc.dram_tensor("maskd", [128, 4, 1024], BF16, kind="ExternalInput")
    d_rotp = nc.dram_tensor("rotp", [128, 128], BF16, kind="ExternalInput")
    d_rotk = nc.dram_tensor("rotk", [64, 128], BF16, kind="ExternalInput")
    d_eyek = nc.dram_tensor("eyek", [64, 128], BF16, kind="ExternalInput")
    d_ones = nc.dram_tensor("onesd", [128, 128], BF16, kind="ExternalInput")
    d_out = nc.dram_tensor("out", [n, EMBED], F32, kind="ExternalOutput")

    with TileContext(nc) as tc, ExitStack() as st:
        # ---- attention-phase persistent tiles (allocated for whole kernel,
        # prefetched during phase 1) ----
        attn_p = st.enter_context(tc.tile_pool(name="attn_p", bufs=1))
        t_qn = attn_p.tile([128, NH, n], BF16)
        t_qpe = attn_p.tile([128, 2, n], BF16)
        t_kn = attn_p.tile([128, NH, n], BF16)
        t_kpe = attn_p.tile([128, n], BF16)
        t_v = attn_p.tile([128, NT, NH * VDIM], BF16)

        # ---- mid pool: phase-1/2 intermediates, freed before attention ----
        mid = st.enter_context(tc.tile_pool(name="mid", bufs=1))
        t_qa = mid.tile([128, RC, n], BF16)
        t_ckv = mid.tile([128, RC, n], BF16)
        t_kpr = mid.tile([64, n], BF16)  # raw k_pe (pre-rope)
        t_cos = mid.tile([128, n], F32)
        t_sin = mid.tile([128, n], F32)
        t_rotp = mid.tile([128, 128], BF16)
        t_rotk = mid.tile([64, 128], BF16)
        t_eyek = mid.tile([64, 128], BF16)
        t_wqb = mid.tile([128, RC, NH * Q_HEAD], BF16)
        t_wk = mid.tile([128, RC, NH * NOPE], BF16)
        t_wv = mid.tile([128, RC, NH * VDIM], BF16)

        # ================= phase 1: qa = x@Wqa, ckv = x@Wkva =============
        ps12_cm = tc.tile_pool(name="ps12", bufs=1, space="PSUM")
        ps12 = ps12_cm.__enter__()
        with tc.tile_pool(name="ph1", bufs=1) as ph1:
            rot_i = [0]

            def ps_any(name):
                tag = "pa" if rot_i[0] % 8 < 4 else "pb"
                rot_i[0] += 1
                return ps12.tile([128, 512], F32, tag=tag, bufs=4, name=name)

            t_wqa = ph1.tile([128, KC, 512], BF16)
            t_wkva = ph1.tile([128, KC, 576], BF16)
            # critical-path loads first, chunked so the first matmuls can
            # start before the full tensors land; spread over engine queues
            # PE pre-warm: dummy matmuls on a zeroed tile keep the HAM
            # clock-gate at full speed until the first input DMAs land
            # (~14us); their accumulation groups are reset by the real
            # phase-1 matmuls (start=True).
            t_warm = ph1.tile([128, 512], BF16)
            nc.vector.memset(t_warm, 0)
            ps_w = ps_any("ps_w")
            for _ in range(50):
                nc.tensor.matmul(ps_w, t_warm[:, 0:128], t_warm,
                                 start=True, stop=True)
            t_xc = [ph1.tile([128, KC, 512], BF16, tag="tx", bufs=2, name="tx")]
            for kk in range(4):
                kks = slice(4 * kk, 4 * kk + 4)
                nc.sync.dma_start(out=t_xc[0][:, kks], in_=d_x.ap()[:, 0, kks])
                nc.scalar.dma_start(out=t_wqa[:, kks], in_=d_wqa.ap()[:, kks])
            for kk in range(2):
                kks = slice(8 * kk, 8 * kk + 8)
                nc.gpsimd.dma_start(out=t_wkva[:, kks], in_=d_wkva.ap()[:, kks])
            nc.gpsimd.dma_start(out=t_wqb, in_=d_wqb.ap())
            nc.gpsimd.dma_start(out=t_wk, in_=d_wk.ap())
            nc.gpsimd.dma_start(out=t_wv, in_=d_wv.ap())
            nc.gpsimd.dma_start(out=t_cos, in_=d_cos.ap())
            nc.gpsimd.dma_start(out=t_sin, in_=d_sin.ap())
            nc.gpsimd.dma_start(out=t_rotp, in_=d_rotp.ap())
            nc.gpsimd.dma_start(out=t_rotk, in_=d_rotk.ap())
            nc.gpsimd.dma_start(out=t_eyek, in_=d_eyek.ap())

            for t in range(TC):
                ts = slice(t * 512, (t + 1) * 512)
                if t + 1 < TC:
                    t_xc.append(
                        ph1.tile([128, KC, 512], BF16, tag="tx", bufs=2, name="tx")
                    )
                    nc.sync.dma_start(out=t_xc[t + 1], in_=d_x.ap()[:, t + 1])
                t_x = t_xc[t]
                # 8 concurrent accumulation groups (one psum bank each) so an
                # arriving x k-piece immediately feeds 8 matmuls
                pss = [ps_any(f"pg{g}") for g in range(8)]
                for k in range(KC):
                    for g in range(8):
                        w = (
                            t_wqa[:, k, g * 128 : (g + 1) * 128]
                            if g < 4
                            else t_wkva[:, k, (g - 4) * 128 : (g - 3) * 128]
                        )
                        nc.tensor.matmul(
                            pss[g],
                            w,
                            t_x[:, k, :],
                            start=(k == 0),
                            stop=(k == KC - 1),
                        )
                for g in range(8):
                    dst = t_qa[:, g, ts] if g < 4 else t_ckv[:, g - 4, ts]
                    if g % 2 == 0:
                        nc.scalar.copy(dst, pss[g])
                    else:
                        nc.vector.tensor_copy(dst, pss[g])
                # k_pe chunk (64 wide)
                ps = ps_any("ps3")
                for k in range(KC):
                    nc.tensor.matmul(
                        ps[:64],
                        t_wkva[:, k, 512:576],
                        t_x[:, k, :],
                        start=(k == 0),
                        stop=(k == KC - 1),
                    )
                nc.scalar.copy(t_kpr[:, ts], ps[:64])

        wop = st.enter_context(tc.tile_pool(name="wop", bufs=1))
        t_wout = wop.tile([128, NH, EMBED], BF16)
        t_mask = wop.tile([128, 4, 1024], BF16)
        t_ones = wop.tile([128, 128], BF16)
        nc.scalar.dma_start(out=t_wout, in_=d_wout.ap())
        nc.scalar.dma_start(out=t_mask, in_=d_mask.ap())
        nc.scalar.dma_start(out=t_ones, in_=d_ones.ap())

        # ==== phase 2: q/k/v projections + rope, interleaved per chunk ====
        with tc.tile_pool(name="ropep", bufs=4) as rp:
            rot_i = [0]

            def ps_any2(name):
                tag = "pa" if rot_i[0] % 8 < 4 else "pb"
                rot_i[0] += 1
                return ps12.tile([128, 512], F32, tag=tag, bufs=4, name=name)

            def rope_q(g, t):
                ts = slice(t * 512, (t + 1) * 512)
                pr = ps_any2("prq")
                nc.tensor.matmul(pr, t_rotp, t_qpe[:, g, ts])
                tt1 = rp.tile([128, 512], F32, tag="tt1")
                tt2 = rp.tile([128, 512], F32, tag="tt2")
                nc.vector.tensor_tensor(tt1, pr, t_sin[:, ts], op=OP.mult)
                nc.vector.tensor_tensor(
                    tt2, t_qpe[:, g, ts], t_cos[:, ts], op=OP.mult
                )
                nc.vector.tensor_tensor(t_qpe[:, g, ts], tt1, tt2, op=OP.add)

            def rope_k(t):
                ts = slice(t * 512, (t + 1) * 512)
                pr = ps_any2("prk")
                pd = ps_any2("pdk")
                nc.tensor.matmul(pr, t_rotk, t_kpr[:, ts])
                nc.tensor.matmul(pd, t_eyek, t_kpr[:, ts])
                tt1 = rp.tile([128, 512], F32, tag="tt1")
                tt2 = rp.tile([128, 512], F32, tag="tt2")
                nc.vector.tensor_tensor(tt1, pr, t_sin[:, ts], op=OP.mult)
                nc.vector.tensor_tensor(tt2, pd, t_cos[:, ts], op=OP.mult)
                nc.vector.tensor_tensor(t_kpe[:, ts], tt1, tt2, op=OP.add)

            for t in range(TC):
                ts = slice(t * 512, (t + 1) * 512)
                for m in range(6):  # q: 4 nope chunks + 2 pe chunks
                    ps = ps_any2("psq")
                    for k in range(RC):
                        nc.tensor.matmul(
                            ps,
                            t_wqb[:, k, m * 128 : (m + 1) * 128],
                            t_qa[:, k, ts],
                            start=(k == 0),
                            stop=(k == RC - 1),
                        )
                    if m < 4:
                        nc.scalar.copy(t_qn[:, m, ts], ps)
                    else:
                        nc.scalar.copy(t_qpe[:, m - 4, ts], ps)
                rope_q(0, t)
                rope_q(1, t)
                for m in range(4):  # k_nope
                    ps = ps_any2("psk")
                    for k in range(RC):
                        nc.tensor.matmul(
                            ps,
                            t_wk[:, k, m * 128 : (m + 1) * 128],
                            t_ckv[:, k, ts],
                            start=(k == 0),
                            stop=(k == RC - 1),
                        )
                    nc.scalar.copy(t_kn[:, m, ts], ps)
                rope_k(t)
                for mt in range(4 * t, 4 * t + 4):  # v, token-major
                    ps = ps_any2("psv")
                    for k in range(RC):
                        nc.tensor.matmul(
                            ps,
                            t_ckv[:, k, mt * 128 : (mt + 1) * 128],
                            t_wv[:, k, :],
                            start=(k == 0),
                            stop=(k == RC - 1),
                        )
                    nc.scalar.copy(t_v[:, mt, :], ps)

        ps12_cm.__exit__(None, None, None)

        # =================== attention + out-proj ========================
        with (
            tc.tile_pool(name="ptp", bufs=1) as ptp,
            tc.tile_pool(name="accp", bufs=1) as accp,
            tc.tile_pool(name="aop", bufs=1) as aop,
            tc.tile_pool(name="smallp", bufs=1) as smallp,
            tc.tile_pool(name="otp", bufs=4) as otp,
            tc.tile_pool(name="psA", bufs=1, space="PSUM") as psA,
        ):
            def sc_tile(name):
                return psA.tile([128, 1024], F32, tag="sc", bufs=2, name=name)

            def av_tile(name):
                return psA.tile([128, 512], F32, tag="av", bufs=4, name=name)

            op_i = [0]

            def op_store(po, tok, fp):
                # copy PSUM -> SBUF (alternate DVE / ACT) then DMA out
                ot = otp.tile([128, 1024], F32, tag="ot", name="ot")
                if op_i[0] % 2 == 0:
                    nc.vector.tensor_copy(ot, po)
                else:
                    nc.scalar.copy(ot, po)
                op_i[0] += 1
                nc.sync.dma_start(
                    out=d_out.ap()[tok : tok + 128, fp * 1024 : (fp + 1) * 1024],
                    in_=ot,
                )

            ao_prev = None  # t_ao tile of the previous query block

            def emit_op_group(mt, fp, qb_tok):
                tok = qb_tok * 512 + mt * 128
                po = sc_tile("po")
                for fh in range(2):
                    f = 2 * fp + fh
                    half = slice(fh * 512, (fh + 1) * 512)
                    for h4 in range(NH):
                        nc.tensor.matmul(
                            po[:, half],
                            ao_prev[:, h4, mt * 128 : (mt + 1) * 128],
                            t_wout[:, h4, f * 512 : (f + 1) * 512],
                            start=(h4 == 0),
                            stop=(h4 == NH - 1),
                        )
                op_store(po, tok, fp)

            for qb in range(TC):
                qs = slice(qb * 512, (qb + 1) * 512)
                nkb = 4 * qb + 4
                ps_av = [av_tile(f"av{h}") for h in range(NH)]
                acc = [accp.tile([128, 1024], BF16, tag=f"acc{p}", bufs=2,
                                 name=f"acc{p}")
                       for p in range(2)]
                t_ao = aop.tile([128, NH, 512], BF16, tag="ao", bufs=2)
                op_work = (
                    [(mt, fp) for mt in range(4) for fp in range(2)]
                    if ao_prev is not None
                    else []
                )
                pending = None
                for kb in range(nkb):
                    ks = slice(kb * 128, (kb + 1) * 128)
                    diag = kb >= 4 * qb
                    r = kb - 4 * qb
                    # --- scores for 4 heads: 2 head-pair tiles ---
                    sc = [sc_tile(f"sc{p}") for p in range(2)]
                    pt = [
                        ptp.tile([128, 1024], BF16, tag=f"pt{p}", bufs=2,
                                 name=f"pt{p}")
                        for p in range(2)
                    ]
                    # for diag chunk r, queries < 128*r are fully masked:
                    # stream only columns [128r, 512). The skipped region of
                    # the psum tile holds stale data; exp reads it and the
                    # mask multiply zeroes it, so downstream stays correct.
                    qoff = 128 * r if diag else 0
                    qsub = slice(qb * 512 + qoff, (qb + 1) * 512)
                    for p in range(2):
                        for hh in range(2):
                            h = 2 * p + hh
                            half = slice(hh * 1024 // 2 + qoff, (hh + 1) * 512)
                            nc.tensor.matmul(
                                sc[p][:, half], t_kn[:, h, ks],
                                t_qn[:, h, qsub],
                                start=True, stop=False,
                                skip_group_check=True,
                            )
                        for hh in range(2):
                            ho = hh * 64
                            half = slice(hh * 512 + qoff, (hh + 1) * 512)
                            nc.tensor.matmul(
                                sc[p][:, half],
                                t_kpe[ho : ho + 64, ks],
                                t_qpe[ho : ho + 64, p, qsub],
                                start=False, stop=True,
                                skip_group_check=True,
                            )
                        nc.scalar.activation(pt[p], sc[p], AF.Exp, scale=SCALE)
                        if diag:
                            nc.vector.tensor_tensor(
                                pt[p], pt[p], t_mask[:, r, :], op=OP.mult
                            )
                        if kb == 0:
                            nc.vector.tensor_copy(acc[p], pt[p])
                        else:
                            nc.vector.tensor_tensor(
                                acc[p], acc[p], pt[p], op=OP.add
                            )
                    # --- interleaved out-proj of previous block ---
                    # two groups on alternate steps: the pair of PSUM tiles
                    # then has a two-step window before the score rotation
                    # reclaims them. Keep two in reserve to cover the den wait
                    # at the block boundary.
                    reserve = 4 if qb == TC - 1 else 2
                    if kb % 3 == 0:
                        for _ in range(2):
                            if len(op_work) > reserve:
                                mt, fp = op_work.pop(0)
                                emit_op_group(mt, fp, qb - 1)
                    # --- attn@V for the previous key chunk (1-step delay) ---
                    if pending is not None:
                        pkb, ppt = pending
                        poff = 128 * (pkb - 4 * qb) if pkb >= 4 * qb else 0
                        for h in range(NH):
                            nc.tensor.matmul(
                                ps_av[h][:, poff:512],
                                t_v[:, pkb, h * VDIM : (h + 1) * VDIM],
                                ppt[h // 2][
                                    :, (h % 2) * 512 + poff : (h % 2 + 1) * 512
                                ],
                                start=(pkb == 0),
                                stop=False,
                                skip_group_check=True,
                            )
                    pending = (kb, pt)
                # flush the last attn@V chunk
                pkb, ppt = pending
                poff = 128 * (pkb - 4 * qb) if pkb >= 4 * qb else 0
                for h in range(NH):
                    nc.tensor.matmul(
                        ps_av[h][:, poff:512],
                        t_v[:, pkb, h * VDIM : (h + 1) * VDIM],
                        ppt[h // 2][:, (h % 2) * 512 + poff : (h % 2 + 1) * 512],
                        start=(pkb == 0),
                        stop=True,
                        skip_group_check=True,
                    )
                # two leftover groups fill the den wait on the acc chain;
                # on the final block two more cover the recip+normalize wait
                for mt, fp in op_work[:2]:
                    emit_op_group(mt, fp, qb - 1)
                op_work = op_work[2:]
                # --- denominators + normalize ---
                # all-ones [128,128] stationary: every PSUM row of the den
                # matmul gets the per-query denominator (free broadcast), then
                # one reciprocal per pair lands 1/den in SBUF.
                ps_den = [sc_tile(f"psden{p}") for p in range(2)]
                if qb == 0:
                    # qb0 has no out-proj work to fill the wait on the final
                    # exp->mask->acc chain; idle here trips the HAM throttle
                    # into half-clock for the next ~7us. Keep the PE busy with
                    # throwaway matmuls whose groups the real den matmuls
                    # reset (start=True) below.
                    for p in range(2):
                        for half in (slice(0, 512), slice(512, 1024)):
                            for _ in range(2):
                                nc.tensor.matmul(
                                    ps_den[p][:, half],
                                    t_ones,
                                    t_qn[:, 0, 0:512],
                                )
                for p in range(2):
                    for half in (slice(0, 512), slice(512, 1024)):
                        nc.tensor.matmul(
                            ps_den[p][:, half],
                            t_ones,
                            acc[p][:, half],
                        )
                bc = [smallp.tile([128, 1024], F32, tag=f"bc{p}", name=f"bc{p}")
                      for p in range(2)]
                nc.vector.reciprocal_approx_fast(bc[0], ps_den[0])
                nc.vector.reciprocal_approx_fast(bc[1], ps_den[1])
                for mt, fp in op_work:  # final-block extra reserve
                    emit_op_group(mt, fp, qb - 1)
                for h in range(NH):
                    nc.vector.tensor_tensor(
                        t_ao[:, h, :],
                        ps_av[h],
                        bc[h // 2][:, (h % 2) * 512 : (h % 2 + 1) * 512],
                        op=OP.mult,
                    )
                ao_prev = t_ao
            # final block's out-projection (tail)
            for mt in range(4):
                for fp in range(2):
                    emit_op_group(mt, fp, TC - 1)
    return nc


_NC_CACHE = {}


def build_mla(n=2048):
    if n not in _NC_CACHE:
        nc = bacc.Bacc(
            "TRN2",
            target_bir_lowering=False,
            debug=False,
            enable_asserts=False,
        )
        _emit(nc, n)
        nc.compile()
        _NC_CACHE[n] = nc
    return _NC_CACHE[n]


def make_host_inputs(x, Wqa, Wqb, Wkva, Wkvb, Wout, n):
    """Build the 8 per-core input maps (host-side sharding)."""
    # rope tables
    theta = BASE ** (-2.0 * np.arange(ROPE // 2, dtype=np.float32) / ROPE)
    pos = np.arange(n, dtype=np.float32)
    ang = pos[:, None] * theta[None, :]  # [n, 32]
    cos64 = np.repeat(np.cos(ang).T, 2, axis=0).astype(np.float32)  # [64, n]
    sin64 = np.repeat(np.sin(ang).T, 2, axis=0).astype(np.float32)
    cosd = np.tile(cos64, (2, 1))  # [128, n]
    sind = np.tile(sin64, (2, 1))

    kp = np.arange(128)[:, None, None]
    r = np.arange(4)[None, :, None]
    qf = np.arange(512)[None, None, :]
    # keep-mask: 1.0 where key <= query (kept), 0.0 where future (excluded)
    keep = (qf >= r * 128 + kp).astype(NPBF16)  # [128, 4, 512]
    maskd = np.concatenate([keep, keep], axis=2)  # [128, 4, 1024] per head-pair

    rot64 = np.zeros((64, 64), np.float32)
    for i in range(32):
        rot64[2 * i + 1, 2 * i] = -1.0
        rot64[2 * i, 2 * i + 1] = 1.0
    rotp = np.zeros((128, 128), np.float32)
    rotp[:64, :64] = rot64
    rotp[64:, 64:] = rot64
    rotk = np.hstack([rot64, rot64])
    eyek = np.hstack([np.eye(64, dtype=np.float32), np.eye(64, dtype=np.float32)])

    def prelay(w, kc):
        # [kc*128, m] -> [128, kc, m] partition-major, contiguous
        return np.ascontiguousarray(
            w.reshape(kc, 128, w.shape[1]).transpose(1, 0, 2)
        ).astype(NPBF16)

    shared = {
        "wqa": prelay(Wqa, KC),
        "wkva": prelay(Wkva, KC),
        "cosd": cosd,
        "sind": sind,
        "maskd": maskd,
        "onesd": np.ones((128, 128), np.float32).astype(NPBF16),
        "rotp": rotp.astype(NPBF16),
        "rotk": rotk.astype(NPBF16),
        "eyek": eyek.astype(NPBF16),
    }
    Wqb_r = Wqb.reshape(512, HEADS, Q_HEAD)
    Wkvb_r = Wkvb.reshape(KV_RANK, HEADS, NOPE + VDIM)
    Wout_r = Wout.reshape(HEADS, VDIM, EMBED)

    in_maps = []
    TC = n // 512
    # x[be].T -> [128, TC, KC, 512]: f=(c,p), t=(tb,tt)
    xT = [
        np.ascontiguousarray(
            x[be].T.reshape(KC, 128, TC, 512).transpose(1, 2, 0, 3)
        ).astype(NPBF16)
        for be in range(x.shape[0])
    ]
    for c in range(8):
        be, hg = c // 4, c % 4
        hsel = slice(4 * hg, 4 * hg + NH)
        wqb = prelay(
            np.concatenate(
                [
                    Wqb_r[:, hsel, :NOPE].reshape(512, NH * NOPE),
                    Wqb_r[:, hsel, NOPE:].reshape(512, NH * ROPE),
                ],
                axis=1,
            ),
            RC,
        )
        in_maps.append(
            {
                **shared,
                "xT": xT[be],
                "wqb": wqb,
                "wk": prelay(Wkvb_r[:, hsel, :NOPE].reshape(512, NH * NOPE), RC),
                "wv": prelay(Wkvb_r[:, hsel, NOPE:].reshape(512, NH * VDIM), RC),
                "wout": prelay(Wout_r[hsel].reshape(NH * VDIM, EMBED), NH),
            }
        )
    return in_maps


def kernel(x, Wqa, Wqb, Wkva, Wkvb, Wout, _trace=False):
    x = np.asarray(x)
    b, n, _ = x.shape
    nc = build_mla(n)
    in_maps = make_host_inputs(
        np.asarray(x),
        np.asarray(Wqa),
        np.asarray(Wqb),
        np.asarray(Wkva),
        np.asarray(Wkvb),
        np.asarray(Wout),
        n,
    )
    res = bass_utils.run_bass_kernel_spmd(
        nc, in_maps, core_ids=list(range(8)), trace=_trace
    )
    out = np.zeros((b, n, EMBED), np.float32)
    for c in range(8):
        out[c // 4] += res.results[c]["out"]
    if _trace:
        kernel.last_results = res
    return out
